# revision 15
# baseline (speedup 1.0000x reference)
"""MoE expert-parallel kernel for Trainium2 (8 NeuronCores).

Problem: nn_Experts (T=8192 tokens, d_model=1024, d_ff=4096, E=8 experts,
top-k=2).  out[t] = sum_e w[t,e] * (relu(x[t] @ wi[e].T) @ wo[e].T), where
w[t,e] is the combined routing weight (0 for unrouted pairs).

Strategy (expert parallelism, sharding_hint):
  - Host: compute w[t,e], gather the tokens routed to each expert e,
    pad to a common capacity C = min(max per-expert count, 1920), and
    ship expert e's tokens + weights to core e.  All-to-all dispatch is
    done host-side; the device kernel is a dense fused 2-layer MLP per
    core.  Tokens beyond the capacity (MoE capacity factor ~1.0) are
    computed exactly on the host in fp32 (~100-300 of 15.4k pairs).
  - Device (per core, SPMD): yT = wo @ relu(wi @ xT) in a transposed,
    weight-stationary formulation.  Weights are converted to bf16 and kept
    resident in SBUF (16.8 MB); matmuls accumulate in fp32 PSUM; y is
    written back as bf16 (halves writeback traffic; host accumulates fp32).
  - Host: scatter-add w[t,e] * y back into the full [T, d_model] output.

Measured numerics (vs fp32 reference): max-abs rel err ~4.4e-3.

Perf notes (measured on this container's trn2 via slope timing):
  - The kernel is PE-streaming-bound: ~0.53 ns per moving-operand column
    sustained (an effective ~1.9 GHz, below the 2.4 GHz PLL - sustained
    power throttling).  Time is invariant to matmul instruction count at
    fixed total columns (TC=256 == TC=512), so per-instruction overheads
    (LDWEIGHTS, NX dispatch, semaphores) are fully hidden.
  - fp8 paths are dead ends here: e4m3 DoubleRow nets only ~1.2-1.4x
    per MAC sustained and raw-fp8 numerics (5.2e-2) blow the 2e-2 gate;
    error-compensated variants cost more columns than they save.  An
    e3m4 moving operand for mm2 measured *slower* (+1.8%) with 4x the
    error.  A same-weights probe (no LDW traffic) measured no gain.
"""
import os
import sys
from contextlib import ExitStack

import numpy as np

sys.path.insert(0, "/opt/trn_rl_repo")

import concourse.bass as bass
import concourse.mybir as mybir
from concourse import tile
from concourse import bass2jax
from concourse.bass2jax import _bass_exec_p, install_neuronx_cc_hook

T, D_MODEL, D_FF, N_EXPERTS, TOP_K = 8192, 1024, 4096, 8, 2
N_CORES = 8
P = 128          # partitions
TC = int(os.environ.get("KTC", "512"))  # token chunk (matmul moving free dim)
KD = D_MODEL // P    # 8 contraction steps for mm1 / output tiles for mm2
MF = D_FF // P       # 32 ff tiles
COMPUTE_DT = mybir.dt.bfloat16
MM_COL_SPLIT = 1   # 1 = plain [K=128,M=128] matmuls; 2/4 = col-group split
# y writeback dtype: bf16 halves PSUM->SBUF->DRAM traffic; host accumulates
# in fp32 so the extra rounding is ~1 ulp of bf16 on y only.
Y_DT = (mybir.dt.bfloat16 if os.environ.get("KYDT", "bf16") == "bf16"
        else mybir.dt.float32)
# device token capacity cap (capacity factor ~1.0 = T*TOP_K/N_EXPERTS):
# tokens beyond this per expert are computed on the host in fp32 (exact).
# 0 = no cap.
CAPACITY = int(os.environ.get("KCAP", "1920"))
# timing-only probe: all matmuls reuse one weight tile (results are wrong);
# isolates the cost of LDWEIGHTS traffic.
SAME_W = int(os.environ.get("KSAMEW", "0"))
# h (mm1 output / mm2 moving operand) dtype: bfloat16 or float8e3 (e3m4).
H_DT = getattr(mybir.dt, os.environ.get("KHDT", "bfloat16"))


def split_multi_waits(nc, max_waits=1):
    """This container's walrus codegen rejects instructions carrying more
    than a couple of semaphore waits (e.g. the TileContext tail Drain).
    Move excess waits onto preceding NoOps on the same engine."""
    for f in nc.m.functions:
        for b in f.blocks:
            il = b.instructions
            i = 0
            while i < len(il):
                inst = il[i]
                si = inst.sync_info
                if si is not None and len(si.on_wait) > max_waits:
                    waits = list(si.on_wait)
                    si.on_wait = waits[:max_waits]
                    inst.sync_info = si
                    pre = []
                    rest = waits[max_waits:]
                    for k in range(0, len(rest), max_waits):
                        nop = mybir.InstNoOp(
                            name=f"{inst.name}-ws-{k}", ins=[], outs=[])
                        nop.engine = inst.engine
                        nop.sync_info = mybir.SyncInfo(
                            on_wait=rest[k:k + max_waits], on_update=[])
                        pre.append(nop)
                    for n in reversed(pre):
                        il.insert(i, n)
                    i += len(pre)
                i += 1


class SpmdRunner:
    """Compile a Bass program once; run it SPMD on n_cores via PJRT/axon."""

    def __init__(self, nc, n_cores):
        import jax
        from jax.sharding import Mesh, PartitionSpec
        from jax.experimental.shard_map import shard_map

        install_neuronx_cc_hook()
        self.nc = nc
        self.n_cores = n_cores
        partition_name = (nc.partition_id_tensor.name
                          if nc.partition_id_tensor else None)
        in_names, out_names, out_avals, zero_outs = [], [], [], []
        for alloc in nc.m.functions[0].allocations:
            if not isinstance(alloc, mybir.MemoryLocationSet):
                continue
            name = alloc.memorylocations[0].name
            if alloc.kind == "ExternalInput":
                if name != partition_name:
                    in_names.append(name)
            elif alloc.kind == "ExternalOutput":
                out_names.append(name)
                shape = tuple(alloc.tensor_shape)
                dtype = mybir.dt.np(alloc.dtype)
                out_avals.append(jax.core.ShapedArray(shape, dtype))
                zero_outs.append(np.zeros(shape, dtype))
        self.in_names = in_names
        self.out_names = out_names
        self.out_avals = out_avals
        self.zero_outs = zero_outs
        n_params = len(in_names)
        n_outs = len(out_avals)
        all_in_names = list(in_names) + list(out_names)
        if partition_name is not None:
            all_in_names.append(partition_name)
        donate = tuple(range(n_params, n_params + n_outs))

        def _body(*args):
            operands = list(args)
            if partition_name is not None:
                operands.append(bass2jax.partition_id_tensor())
            outs = _bass_exec_p.bind(
                *operands,
                out_avals=tuple(out_avals),
                in_names=tuple(all_in_names),
                out_names=tuple(out_names),
                lowering_input_output_aliases=(),
                sim_require_finite=True,
                sim_require_nnan=True,
                nc=nc,
            )
            return tuple(outs)

        devices = jax.devices()[:n_cores]
        assert len(devices) == n_cores, (
            f"need {n_cores} neuron cores, found {len(jax.devices())}")
        mesh = Mesh(np.asarray(devices), ("core",))
        self.mesh = mesh
        in_specs = (PartitionSpec("core"),) * (n_params + n_outs)
        out_specs = (PartitionSpec("core"),) * n_outs
        self.sharded = jax.jit(
            shard_map(_body, mesh=mesh, in_specs=in_specs,
                      out_specs=out_specs, check_rep=False),
            donate_argnums=donate, keep_unused=True)

    def prep(self, in_maps):
        n = self.n_cores
        concat_in = [
            np.concatenate([np.asarray(in_maps[c][name]) for c in range(n)],
                           axis=0)
            for name in self.in_names
        ]
        concat_zeros = self.device_zeros()
        return concat_in, concat_zeros

    def device_zeros(self):
        """Donated output buffers, created directly on device (no H2D)."""
        import jax
        import jax.numpy as jnp
        from jax.sharding import NamedSharding, PartitionSpec
        if not hasattr(self, "_zeros_fn"):
            n = self.n_cores
            shapes = [(n * z.shape[0], *z.shape[1:]) for z in self.zero_outs]
            dts = [z.dtype for z in self.zero_outs]
            sh = tuple(NamedSharding(self.mesh, PartitionSpec("core"))
                       for _ in shapes)
            self._zeros_fn = jax.jit(
                lambda: tuple(jnp.zeros(s, d) for s, d in zip(shapes, dts)),
                out_shardings=sh)
        return list(self._zeros_fn())

    def run_prepped(self, concat_in, concat_zeros=None):
        if concat_zeros is None:
            concat_zeros = self.device_zeros()
        return self.sharded(*concat_in, *concat_zeros)

    def __call__(self, in_maps):
        out_arrs = self.run_prepped(*self.prep(in_maps))
        n = self.n_cores
        return [
            {name: np.asarray(out_arrs[i]).reshape(
                n, *self.out_avals[i].shape)[c]
             for i, name in enumerate(self.out_names)}
            for c in range(n)
        ]


def chunk_widths(C):
    """Split C tokens into near-equal chunks of width <= TC (multiple of 8),
    to avoid padding the capacity all the way up to a TC multiple."""
    n = -(-C // TC)
    w = -(-(-(-C // n)) // 8) * 8
    widths = [w] * (n - 1) + [C - w * (n - 1)]
    assert all(0 < x <= TC for x in widths) and sum(widths) == C, (C, widths)
    return widths


def build_nc(C, n_repeat=1):
    """Per-core fused MLP: yT[:, :C] = wo @ relu(wi @ xT[:, :C]).

    Inputs (per core): xT [D_MODEL, C] bf16, wiT [D_MODEL, D_FF] bf16
    (= wi[e].T), woT [D_FF, D_MODEL] bf16 (= wo[e].T).
    Output: yT [D_MODEL, C] fp32.
    n_repeat>1 wraps the token-chunk sweep in a hardware loop (for slope
    timing; the result is identical each iteration)."""
    assert C % 8 == 0
    widths = chunk_widths(C)
    starts = [sum(widths[:i]) for i in range(len(widths))]
    nchunk = len(widths)
    TCW = widths[0]
    nc = bass.Bass()
    # x is packed chunk-major by the host: chunk c occupies rows
    # [c*D_MODEL, (c+1)*D_MODEL), columns [0, widths[c]) — every per-tile
    # DMA is then a contiguous block instead of C-strided lines.
    xT = nc.declare_dram_parameter("xT", [nchunk * D_MODEL, TCW],
                                   COMPUTE_DT, isOutput=False)
    wiT = nc.declare_dram_parameter("wiT", [D_MODEL, D_FF], COMPUTE_DT,
                                    isOutput=False)
    woT = nc.declare_dram_parameter("woT", [D_FF, D_MODEL], COMPUTE_DT,
                                    isOutput=False)
    yT = nc.declare_dram_parameter("yT", [D_MODEL, C], Y_DT, isOutput=True)

    with ExitStack() as ctx:
        tc = ctx.enter_context(tile.TileContext(nc))
        wpool = ctx.enter_context(tc.tile_pool(name="w", bufs=1))
        xpool = ctx.enter_context(tc.tile_pool(name="x", bufs=1))
        hpool = ctx.enter_context(tc.tile_pool(name="h", bufs=1))
        ypool = ctx.enter_context(tc.tile_pool(name="y", bufs=4))
        pspool = ctx.enter_context(
            tc.tile_pool(name="ps", bufs=8, space="PSUM"))

        # x for chunk 0 first, then wi in ff-quarters (mm1's m-groups need
        # quarter m//8 only), then wo — so the first matmuls start after
        # ~3 MB of DMA instead of the full 16.8 MB of weights.  All x
        # chunks are SBUF-resident (~4 MB) so the repeat loop re-reads
        # nothing from DRAM.
        def x_dma(t, c, k, w):
            r0 = c * D_MODEL + k * P
            nc.sync.dma_start(out=t[:], in_=xT[r0:r0 + P, 0:w])

        x_all = [[None] * KD for _ in range(nchunk)]
        for k in range(KD):
            t = xpool.tile([P, widths[0]], COMPUTE_DT, tag=f"x{k}_0")
            x_dma(t, 0, k, widths[0])
            x_all[0][k] = t
        # first 128 ff-columns of wi separately (m=0 starts after 1.25 MB of
        # DMA instead of 3 MB), then the quarters
        wi_first = []
        for k in range(KD):
            t = wpool.tile([P, P], COMPUTE_DT, tag=f"wif{k}")
            nc.sync.dma_start(out=t[:], in_=wiT[k * P:(k + 1) * P, 0:P])
            wi_first.append(t)
        NQ = 4
        QF = D_FF // NQ
        wi_t = [[None] * NQ for _ in range(KD)]
        for q in range(NQ):
            for k in range(KD):
                t = wpool.tile([P, QF], COMPUTE_DT, tag=f"wi{k}_{q}")
                nc.sync.dma_start(
                    out=t[:], in_=wiT[k * P:(k + 1) * P,
                                      q * QF:(q + 1) * QF])
                wi_t[k][q] = t
        wo_t = []
        for m in range(MF):
            t = wpool.tile([P, D_MODEL], COMPUTE_DT, tag=f"wo{m}")
            nc.sync.dma_start(out=t[:], in_=woT[m * P:(m + 1) * P, :])
            wo_t.append(t)
        for c in range(1, nchunk):
            for k in range(KD):
                t = xpool.tile([P, widths[c]], COMPUTE_DT, tag=f"x{k}_{c}")
                x_dma(t, c, k, widths[c])
                x_all[c][k] = t

        def chunk_sweep(first=False):
            for c, (c0, w) in enumerate(zip(starts, widths)):
                sl = slice(c0, c0 + w)
                x_t = x_all[c]
                def mm(ps, lhsT_full, rhs, start, stop):
                    if MM_COL_SPLIT == 1:
                        nc.tensor.matmul(ps[:], lhsT_full, rhs,
                                         start=start, stop=stop)
                    else:
                        MS = P // MM_COL_SPLIT
                        for s in range(MM_COL_SPLIT):
                            nc.tensor.matmul(
                                ps[s * MS:(s + 1) * MS, :],
                                lhsT_full[:, s * MS:(s + 1) * MS], rhs,
                                start=start, stop=stop,
                                tile_position=(0, s * MS))

                h_t = []
                for m in range(MF):
                    q, mq = divmod(m, MF // NQ)
                    ps = pspool.tile([P, w], mybir.dt.float32, tag="ps")
                    for k in range(KD):
                        if SAME_W:
                            lhsT = wi_first[0][:]
                        elif first and c == 0 and m == 0:
                            lhsT = wi_first[k][:]
                        else:
                            lhsT = wi_t[k][q][:, mq * P:(mq + 1) * P]
                        mm(ps, lhsT, x_t[k][:], k == 0, k == KD - 1)
                    h = hpool.tile([P, w], H_DT, tag=f"h{m}")
                    nc.scalar.activation(h[:], ps[:],
                                         mybir.ActivationFunctionType.Relu)
                    h_t.append(h)
                for n in range(KD):
                    ps = pspool.tile([P, w], mybir.dt.float32, tag="ps")
                    for m in range(MF):
                        mm(ps,
                           wo_t[0][:, 0:P] if SAME_W else
                           wo_t[m][:, n * P:(n + 1) * P], h_t[m][:],
                           m == 0, m == MF - 1)
                    y = ypool.tile([P, w], Y_DT, tag="y")
                    nc.vector.tensor_copy(y[:], ps[:])
                    nc.sync.dma_start(out=yT[n * P:(n + 1) * P, sl],
                                      in_=y[:])

        if n_repeat == 1:
            chunk_sweep(first=True)
        else:
            with tc.For_i(0, n_repeat, 1,
                          hint_engines=(mybir.EngineType.PE,)):
                chunk_sweep()

    split_multi_waits(nc)
    return nc


_RUNNERS = {}


def _get_runner(C, n_repeat=1):
    key = (C, n_repeat)
    if key not in _RUNNERS:
        _RUNNERS[key] = SpmdRunner(build_nc(C, n_repeat), N_CORES)
    return _RUNNERS[key]


def _route(hidden_states, selected_experts, routing_weights):
    """Combined per-token weight for each expert and per-expert token lists."""
    mask = selected_experts.astype(np.float32)          # [T, K, E]
    w_te = np.einsum('tke,tk->te', mask, routing_weights.astype(np.float32))
    idx = [np.nonzero(w_te[:, e] > 0)[0] for e in range(N_EXPERTS)]
    return w_te, idx


def to_bf16(a):
    """Vectorized fp32 -> bf16 cast (round-to-nearest-even), ~3x faster
    than ml_dtypes astype.  Matches ml_dtypes/hardware rounding for finite
    values (inputs here are well-scaled gaussians)."""
    import ml_dtypes
    a = np.ascontiguousarray(a, dtype=np.float32)
    u = a.view(np.uint32)
    r = ((u + 0x7FFF + ((u >> 16) & 1)) >> 16).astype(np.uint16)
    return r.view(ml_dtypes.bfloat16).reshape(a.shape)


def pack_x(hidden_states, ie, C):
    """Per-core x, chunk-major: [nchunk*D_MODEL, widths[0]] bf16 so every
    in-kernel x DMA is a contiguous block."""
    import ml_dtypes
    widths = chunk_widths(C)
    TCW = widths[0]
    xg = to_bf16(hidden_states[ie].transpose(1, 0))      # [D_MODEL, n]
    out = np.zeros((len(widths) * D_MODEL, TCW), dtype=ml_dtypes.bfloat16)
    c0 = 0
    for c, w in enumerate(widths):
        seg = xg[:, c0:min(c0 + w, xg.shape[1])]
        out[c * D_MODEL:(c + 1) * D_MODEL, :seg.shape[1]] = seg
        c0 += w
    return out


# Cached device-resident weight uploads: [(wi_copy, wo_copy, C, dev_arrays)]
_WEIGHT_CACHE = []


def _pack_weights(wi, wo, runner):
    """bf16-pack and upload the per-expert transposed weights once; reuse the
    device arrays on later calls with identical weights."""
    import jax
    for cwi, cwo, cC, dev in _WEIGHT_CACHE:
        if cC == runner.key_C and np.array_equal(cwi, wi) and \
                np.array_equal(cwo, wo):
            return dev
    wiT = np.concatenate(
        [to_bf16(np.ascontiguousarray(wi[e].transpose(1, 0)))
         for e in range(N_EXPERTS)], axis=0)      # [8*1024, 4096]
    woT = np.concatenate(
        [to_bf16(np.ascontiguousarray(wo[e].transpose(1, 0)))
         for e in range(N_EXPERTS)], axis=0)      # [8*4096, 1024]
    dev = {"wiT": jax.device_put(wiT), "woT": jax.device_put(woT)}
    jax.block_until_ready(list(dev.values()))
    _WEIGHT_CACHE.append((wi.copy(), wo.copy(), runner.key_C, dev))
    del _WEIGHT_CACHE[:-2]
    return dev


def _capacity(idx):
    """Device token capacity: the max per-expert count, rounded up to 8,
    optionally clipped to CAPACITY (overflow tokens go to the host)."""
    max_count = max(len(i) for i in idx)
    C = max(8, ((max_count + 7) // 8) * 8)
    if CAPACITY:
        C = min(C, max(8, ((CAPACITY + 7) // 8) * 8))
    return C


def kernel(hidden_states, selected_experts, routing_weights, wi, wo):
    hidden_states = np.asarray(hidden_states)
    selected_experts = np.asarray(selected_experts)
    routing_weights = np.asarray(routing_weights)
    wi = np.asarray(wi)
    wo = np.asarray(wo)

    w_te, idx = _route(hidden_states, selected_experts, routing_weights)
    C = _capacity(idx)
    idx_dev = [ie[:C] for ie in idx]
    idx_spill = [ie[C:] for ie in idx]
    runner = _get_runner(C)
    runner.key_C = C
    wdev = _pack_weights(wi, wo, runner)

    xT = np.concatenate(
        [pack_x(hidden_states, idx_dev[e], C) for e in range(N_EXPERTS)],
        axis=0)
    concat_in = [{"xT": xT, "wiT": wdev["wiT"], "woT": wdev["woT"]}[name]
                 for name in runner.in_names]

    out_arrs = runner.run_prepped(concat_in)
    yT_all = np.asarray(out_arrs[0]).astype(np.float32).reshape(
        N_EXPERTS, D_MODEL, C)

    out = np.zeros((T, D_MODEL), dtype=np.float32)
    for e in range(N_EXPERTS):
        ie = idx_dev[e]
        out[ie] += w_te[ie, e:e + 1] * yT_all[e, :, :len(ie)].T
        sp = idx_spill[e]
        if len(sp):
            h = np.maximum(hidden_states[sp] @ wi[e].T, 0.0)
            out[sp] += w_te[sp, e:e + 1] * (h @ wo[e].T)
    return out



# revision 16
# speedup vs baseline: 1.0181x; 1.0181x over previous
"""MoE expert-parallel kernel for Trainium2 (8 NeuronCores).

Problem: nn_Experts (T=8192 tokens, d_model=1024, d_ff=4096, E=8 experts,
top-k=2).  out[t] = sum_e w[t,e] * (relu(x[t] @ wi[e].T) @ wo[e].T), where
w[t,e] is the combined routing weight (0 for unrouted pairs).

Strategy (expert parallelism, sharding_hint):
  - Host: compute w[t,e], gather the tokens routed to each expert e,
    pad to a common capacity C = min(max per-expert count, 1920), and
    ship expert e's tokens + weights to core e.  All-to-all dispatch is
    done host-side; the device kernel is a dense fused 2-layer MLP per
    core.  Tokens beyond the capacity (MoE capacity factor ~1.0) are
    computed exactly on the host in fp32 (~100-300 of 15.4k pairs).
  - Device (per core, SPMD): yT = wo @ relu(wi @ xT) in a transposed,
    weight-stationary formulation.  Weights are converted to bf16 and kept
    resident in SBUF (16.8 MB); matmuls accumulate in fp32 PSUM; y is
    written back as bf16 (halves writeback traffic; host accumulates fp32).
  - Host: scatter-add w[t,e] * y back into the full [T, d_model] output.

Measured numerics (vs fp32 reference): max-abs rel err ~4.4e-3.

Perf notes (measured on this container's trn2 via slope timing):
  - The kernel is PE-streaming-bound: ~0.53 ns per moving-operand column
    sustained (an effective ~1.9 GHz, below the 2.4 GHz PLL - sustained
    power throttling).  Time is invariant to matmul instruction count at
    fixed total columns (TC=256 == TC=512), so per-instruction overheads
    (LDWEIGHTS, NX dispatch, semaphores) are fully hidden.
  - fp8 paths are dead ends here: e4m3 DoubleRow nets only ~1.2-1.4x
    per MAC sustained and raw-fp8 numerics (5.2e-2) blow the 2e-2 gate;
    error-compensated variants cost more columns than they save.  An
    e3m4 moving operand for mm2 measured *slower* (+1.8%) with 4x the
    error.  A same-weights probe (no LDW traffic) measured no gain.
"""
import os
import sys
from contextlib import ExitStack

import numpy as np

sys.path.insert(0, "/opt/trn_rl_repo")

import concourse.bass as bass
import concourse.mybir as mybir
from concourse import tile
from concourse import bass2jax
from concourse.bass2jax import _bass_exec_p, install_neuronx_cc_hook

T, D_MODEL, D_FF, N_EXPERTS, TOP_K = 8192, 1024, 4096, 8, 2
N_CORES = 8
P = 128          # partitions
TC = int(os.environ.get("KTC", "512"))  # token chunk (matmul moving free dim)
KD = D_MODEL // P    # 8 contraction steps for mm1 / output tiles for mm2
MF = D_FF // P       # 32 ff tiles
COMPUTE_DT = mybir.dt.bfloat16
MM_COL_SPLIT = 1   # 1 = plain [K=128,M=128] matmuls; 2/4 = col-group split
# y writeback dtype: bf16 halves PSUM->SBUF->DRAM traffic; host accumulates
# in fp32 so the extra rounding is ~1 ulp of bf16 on y only.
Y_DT = (mybir.dt.bfloat16 if os.environ.get("KYDT", "bf16") == "bf16"
        else mybir.dt.float32)
# device token capacity cap (capacity factor ~1.0 = T*TOP_K/N_EXPERTS):
# tokens beyond this per expert are computed on the host in fp32 (exact).
# 0 = no cap.
CAPACITY = int(os.environ.get("KCAP", "1920"))
# timing-only probe: all matmuls reuse one weight tile (results are wrong);
# isolates the cost of LDWEIGHTS traffic.
SAME_W = int(os.environ.get("KSAMEW", "0"))
# h (mm1 output / mm2 moving operand) dtype: bfloat16 or float8e3 (e3m4).
H_DT = getattr(mybir.dt, os.environ.get("KHDT", "bfloat16"))


def split_multi_waits(nc, max_waits=1):
    """This container's walrus codegen rejects instructions carrying more
    than a couple of semaphore waits (e.g. the TileContext tail Drain).
    Move excess waits onto preceding NoOps on the same engine."""
    for f in nc.m.functions:
        for b in f.blocks:
            il = b.instructions
            i = 0
            while i < len(il):
                inst = il[i]
                si = inst.sync_info
                if si is not None and len(si.on_wait) > max_waits:
                    waits = list(si.on_wait)
                    si.on_wait = waits[:max_waits]
                    inst.sync_info = si
                    pre = []
                    rest = waits[max_waits:]
                    for k in range(0, len(rest), max_waits):
                        nop = mybir.InstNoOp(
                            name=f"{inst.name}-ws-{k}", ins=[], outs=[])
                        nop.engine = inst.engine
                        nop.sync_info = mybir.SyncInfo(
                            on_wait=rest[k:k + max_waits], on_update=[])
                        pre.append(nop)
                    for n in reversed(pre):
                        il.insert(i, n)
                    i += len(pre)
                i += 1


class SpmdRunner:
    """Compile a Bass program once; run it SPMD on n_cores via PJRT/axon."""

    def __init__(self, nc, n_cores):
        import jax
        from jax.sharding import Mesh, PartitionSpec
        from jax.experimental.shard_map import shard_map

        install_neuronx_cc_hook()
        self.nc = nc
        self.n_cores = n_cores
        partition_name = (nc.partition_id_tensor.name
                          if nc.partition_id_tensor else None)
        in_names, out_names, out_avals, zero_outs = [], [], [], []
        for alloc in nc.m.functions[0].allocations:
            if not isinstance(alloc, mybir.MemoryLocationSet):
                continue
            name = alloc.memorylocations[0].name
            if alloc.kind == "ExternalInput":
                if name != partition_name:
                    in_names.append(name)
            elif alloc.kind == "ExternalOutput":
                out_names.append(name)
                shape = tuple(alloc.tensor_shape)
                dtype = mybir.dt.np(alloc.dtype)
                out_avals.append(jax.core.ShapedArray(shape, dtype))
                zero_outs.append(np.zeros(shape, dtype))
        self.in_names = in_names
        self.out_names = out_names
        self.out_avals = out_avals
        self.zero_outs = zero_outs
        n_params = len(in_names)
        n_outs = len(out_avals)
        all_in_names = list(in_names) + list(out_names)
        if partition_name is not None:
            all_in_names.append(partition_name)
        donate = tuple(range(n_params, n_params + n_outs))

        def _body(*args):
            operands = list(args)
            if partition_name is not None:
                operands.append(bass2jax.partition_id_tensor())
            outs = _bass_exec_p.bind(
                *operands,
                out_avals=tuple(out_avals),
                in_names=tuple(all_in_names),
                out_names=tuple(out_names),
                lowering_input_output_aliases=(),
                sim_require_finite=True,
                sim_require_nnan=True,
                nc=nc,
            )
            return tuple(outs)

        devices = jax.devices()[:n_cores]
        assert len(devices) == n_cores, (
            f"need {n_cores} neuron cores, found {len(jax.devices())}")
        mesh = Mesh(np.asarray(devices), ("core",))
        self.mesh = mesh
        in_specs = (PartitionSpec("core"),) * (n_params + n_outs)
        out_specs = (PartitionSpec("core"),) * n_outs
        self.sharded = jax.jit(
            shard_map(_body, mesh=mesh, in_specs=in_specs,
                      out_specs=out_specs, check_rep=False),
            donate_argnums=donate, keep_unused=True)

    def prep(self, in_maps):
        n = self.n_cores
        concat_in = [
            np.concatenate([np.asarray(in_maps[c][name]) for c in range(n)],
                           axis=0)
            for name in self.in_names
        ]
        concat_zeros = self.device_zeros()
        return concat_in, concat_zeros

    def device_zeros(self):
        """Donated output buffers, created directly on device (no H2D)."""
        import jax
        import jax.numpy as jnp
        from jax.sharding import NamedSharding, PartitionSpec
        if not hasattr(self, "_zeros_fn"):
            n = self.n_cores
            shapes = [(n * z.shape[0], *z.shape[1:]) for z in self.zero_outs]
            dts = [z.dtype for z in self.zero_outs]
            sh = tuple(NamedSharding(self.mesh, PartitionSpec("core"))
                       for _ in shapes)
            self._zeros_fn = jax.jit(
                lambda: tuple(jnp.zeros(s, d) for s, d in zip(shapes, dts)),
                out_shardings=sh)
        return list(self._zeros_fn())

    def run_prepped(self, concat_in, concat_zeros=None):
        if concat_zeros is None:
            concat_zeros = self.device_zeros()
        return self.sharded(*concat_in, *concat_zeros)

    def __call__(self, in_maps):
        out_arrs = self.run_prepped(*self.prep(in_maps))
        n = self.n_cores
        return [
            {name: np.asarray(out_arrs[i]).reshape(
                n, *self.out_avals[i].shape)[c]
             for i, name in enumerate(self.out_names)}
            for c in range(n)
        ]


def chunk_widths(C):
    """Split C tokens into near-equal chunks of width <= TC (multiple of 8),
    to avoid padding the capacity all the way up to a TC multiple."""
    n = -(-C // TC)
    w = -(-(-(-C // n)) // 8) * 8
    widths = [w] * (n - 1) + [C - w * (n - 1)]
    assert all(0 < x <= TC for x in widths) and sum(widths) == C, (C, widths)
    return widths


def build_nc(C, n_repeat=1):
    """Per-core fused MLP: yT[:, :C] = wo @ relu(wi @ xT[:, :C]).

    Inputs (per core): xT [D_MODEL, C] bf16, wiT [D_MODEL, D_FF] bf16
    (= wi[e].T), woT [D_FF, D_MODEL] bf16 (= wo[e].T).
    Output: yT [D_MODEL, C] fp32.
    n_repeat>1 wraps the token-chunk sweep in a hardware loop (for slope
    timing; the result is identical each iteration)."""
    assert C % 8 == 0
    widths = chunk_widths(C)
    starts = [sum(widths[:i]) for i in range(len(widths))]
    nchunk = len(widths)
    TCW = widths[0]
    nc = bass.Bass()
    # x is packed chunk-major by the host: chunk c occupies rows
    # [c*D_MODEL, (c+1)*D_MODEL), columns [0, widths[c]) — every per-tile
    # DMA is then a contiguous block instead of C-strided lines.
    xT = nc.declare_dram_parameter("xT", [nchunk * D_MODEL, TCW],
                                   COMPUTE_DT, isOutput=False)
    wiT = nc.declare_dram_parameter("wiT", [D_MODEL, D_FF], COMPUTE_DT,
                                    isOutput=False)
    woT = nc.declare_dram_parameter("woT", [D_FF, D_MODEL], COMPUTE_DT,
                                    isOutput=False)
    yT = nc.declare_dram_parameter("yT", [D_MODEL, C], Y_DT, isOutput=True)

    with ExitStack() as ctx:
        tc = ctx.enter_context(tile.TileContext(nc))
        wpool = ctx.enter_context(tc.tile_pool(name="w", bufs=1))
        xpool = ctx.enter_context(tc.tile_pool(name="x", bufs=2))
        hpool = ctx.enter_context(tc.tile_pool(name="h", bufs=1))
        ypool = ctx.enter_context(tc.tile_pool(name="y", bufs=4))
        pspool = ctx.enter_context(
            tc.tile_pool(name="ps", bufs=8, space="PSUM"))

        # x for chunk 0 first, then wi in ff-quarters (mm1's m-groups need
        # quarter m//8 only), then wo — so the first matmuls start after
        # ~3 MB of DMA instead of the full 16.8 MB of weights.
        def x_dma(t, c, k, w):
            r0 = c * D_MODEL + k * P
            nc.sync.dma_start(out=t[:], in_=xT[r0:r0 + P, 0:w])

        x0_t = []
        for k in range(KD):
            t = xpool.tile([P, widths[0]], COMPUTE_DT, tag=f"x{k}")
            x_dma(t, 0, k, widths[0])
            x0_t.append(t)
        # first 128 ff-columns of wi separately (m=0 starts after 1.25 MB of
        # DMA instead of 3 MB), then the quarters
        wi_first = []
        for k in range(KD):
            t = wpool.tile([P, P], COMPUTE_DT, tag=f"wif{k}")
            nc.sync.dma_start(out=t[:], in_=wiT[k * P:(k + 1) * P, 0:P])
            wi_first.append(t)
        NQ = 4
        QF = D_FF // NQ
        wi_t = [[None] * NQ for _ in range(KD)]
        for q in range(NQ):
            for k in range(KD):
                t = wpool.tile([P, QF], COMPUTE_DT, tag=f"wi{k}_{q}")
                nc.sync.dma_start(
                    out=t[:], in_=wiT[k * P:(k + 1) * P,
                                      q * QF:(q + 1) * QF])
                wi_t[k][q] = t
        wo_t = []
        for m in range(MF):
            t = wpool.tile([P, D_MODEL], COMPUTE_DT, tag=f"wo{m}")
            nc.sync.dma_start(out=t[:], in_=woT[m * P:(m + 1) * P, :])
            wo_t.append(t)

        def chunk_sweep(first=False):
            for c, (c0, w) in enumerate(zip(starts, widths)):
                sl = slice(c0, c0 + w)
                if first and c == 0:
                    x_t = x0_t
                else:
                    x_t = []
                    for k in range(KD):
                        t = xpool.tile([P, w], COMPUTE_DT, tag=f"x{k}")
                        x_dma(t, c, k, w)
                        x_t.append(t)
                def mm(ps, lhsT_full, rhs, start, stop):
                    if MM_COL_SPLIT == 1:
                        nc.tensor.matmul(ps[:], lhsT_full, rhs,
                                         start=start, stop=stop)
                    else:
                        MS = P // MM_COL_SPLIT
                        for s in range(MM_COL_SPLIT):
                            nc.tensor.matmul(
                                ps[s * MS:(s + 1) * MS, :],
                                lhsT_full[:, s * MS:(s + 1) * MS], rhs,
                                start=start, stop=stop,
                                tile_position=(0, s * MS))

                h_t = []
                for m in range(MF):
                    q, mq = divmod(m, MF // NQ)
                    ps = pspool.tile([P, w], mybir.dt.float32, tag="ps")
                    for k in range(KD):
                        if SAME_W:
                            lhsT = wi_first[0][:]
                        elif first and c == 0 and m == 0:
                            lhsT = wi_first[k][:]
                        else:
                            lhsT = wi_t[k][q][:, mq * P:(mq + 1) * P]
                        mm(ps, lhsT, x_t[k][:], k == 0, k == KD - 1)
                    h = hpool.tile([P, w], H_DT, tag=f"h{m}")
                    nc.scalar.activation(h[:], ps[:],
                                         mybir.ActivationFunctionType.Relu)
                    h_t.append(h)
                for n in range(KD):
                    ps = pspool.tile([P, w], mybir.dt.float32, tag="ps")
                    for m in range(MF):
                        mm(ps,
                           wo_t[0][:, 0:P] if SAME_W else
                           wo_t[m][:, n * P:(n + 1) * P], h_t[m][:],
                           m == 0, m == MF - 1)
                    y = ypool.tile([P, w], Y_DT, tag="y")
                    nc.vector.tensor_copy(y[:], ps[:])
                    nc.sync.dma_start(out=yT[n * P:(n + 1) * P, sl],
                                      in_=y[:])

        if n_repeat == 1:
            chunk_sweep(first=True)
        else:
            with tc.For_i(0, n_repeat, 1,
                          hint_engines=(mybir.EngineType.PE,)):
                chunk_sweep()

    split_multi_waits(nc)
    return nc


_RUNNERS = {}


def _get_runner(C, n_repeat=1):
    key = (C, n_repeat)
    if key not in _RUNNERS:
        _RUNNERS[key] = SpmdRunner(build_nc(C, n_repeat), N_CORES)
    return _RUNNERS[key]


def _route(hidden_states, selected_experts, routing_weights):
    """Combined per-token weight for each expert and per-expert token lists."""
    mask = selected_experts.astype(np.float32)          # [T, K, E]
    w_te = np.einsum('tke,tk->te', mask, routing_weights.astype(np.float32))
    idx = [np.nonzero(w_te[:, e] > 0)[0] for e in range(N_EXPERTS)]
    return w_te, idx


def to_bf16(a):
    """Vectorized fp32 -> bf16 cast (round-to-nearest-even), ~3x faster
    than ml_dtypes astype.  Matches ml_dtypes/hardware rounding for finite
    values (inputs here are well-scaled gaussians)."""
    import ml_dtypes
    a = np.ascontiguousarray(a, dtype=np.float32)
    u = a.view(np.uint32)
    r = ((u + 0x7FFF + ((u >> 16) & 1)) >> 16).astype(np.uint16)
    return r.view(ml_dtypes.bfloat16).reshape(a.shape)


def pack_x(hidden_states, ie, C):
    """Per-core x, chunk-major: [nchunk*D_MODEL, widths[0]] bf16 so every
    in-kernel x DMA is a contiguous block."""
    import ml_dtypes
    widths = chunk_widths(C)
    TCW = widths[0]
    xg = to_bf16(hidden_states[ie].transpose(1, 0))      # [D_MODEL, n]
    out = np.zeros((len(widths) * D_MODEL, TCW), dtype=ml_dtypes.bfloat16)
    c0 = 0
    for c, w in enumerate(widths):
        seg = xg[:, c0:min(c0 + w, xg.shape[1])]
        out[c * D_MODEL:(c + 1) * D_MODEL, :seg.shape[1]] = seg
        c0 += w
    return out


# Cached device-resident weight uploads: [(wi_copy, wo_copy, C, dev_arrays)]
_WEIGHT_CACHE = []


def _pack_weights(wi, wo, runner):
    """bf16-pack and upload the per-expert transposed weights once; reuse the
    device arrays on later calls with identical weights."""
    import jax
    for cwi, cwo, cC, dev in _WEIGHT_CACHE:
        if cC == runner.key_C and np.array_equal(cwi, wi) and \
                np.array_equal(cwo, wo):
            return dev
    wiT = np.concatenate(
        [to_bf16(np.ascontiguousarray(wi[e].transpose(1, 0)))
         for e in range(N_EXPERTS)], axis=0)      # [8*1024, 4096]
    woT = np.concatenate(
        [to_bf16(np.ascontiguousarray(wo[e].transpose(1, 0)))
         for e in range(N_EXPERTS)], axis=0)      # [8*4096, 1024]
    dev = {"wiT": jax.device_put(wiT), "woT": jax.device_put(woT)}
    jax.block_until_ready(list(dev.values()))
    _WEIGHT_CACHE.append((wi.copy(), wo.copy(), runner.key_C, dev))
    del _WEIGHT_CACHE[:-2]
    return dev


def _capacity(idx):
    """Device token capacity: the max per-expert count, rounded up to 8,
    optionally clipped to CAPACITY (overflow tokens go to the host)."""
    max_count = max(len(i) for i in idx)
    C = max(8, ((max_count + 7) // 8) * 8)
    if CAPACITY:
        C = min(C, max(8, ((CAPACITY + 7) // 8) * 8))
    return C


def kernel(hidden_states, selected_experts, routing_weights, wi, wo):
    hidden_states = np.asarray(hidden_states)
    selected_experts = np.asarray(selected_experts)
    routing_weights = np.asarray(routing_weights)
    wi = np.asarray(wi)
    wo = np.asarray(wo)

    w_te, idx = _route(hidden_states, selected_experts, routing_weights)
    C = _capacity(idx)
    idx_dev = [ie[:C] for ie in idx]
    idx_spill = [ie[C:] for ie in idx]
    runner = _get_runner(C)
    runner.key_C = C
    wdev = _pack_weights(wi, wo, runner)

    xT = np.concatenate(
        [pack_x(hidden_states, idx_dev[e], C) for e in range(N_EXPERTS)],
        axis=0)
    concat_in = [{"xT": xT, "wiT": wdev["wiT"], "woT": wdev["woT"]}[name]
                 for name in runner.in_names]

    out_arrs = runner.run_prepped(concat_in)
    yT_all = np.asarray(out_arrs[0]).astype(np.float32).reshape(
        N_EXPERTS, D_MODEL, C)

    out = np.zeros((T, D_MODEL), dtype=np.float32)
    for e in range(N_EXPERTS):
        ie = idx_dev[e]
        out[ie] += w_te[ie, e:e + 1] * yT_all[e, :, :len(ie)].T
        sp = idx_spill[e]
        if len(sp):
            h = np.maximum(hidden_states[sp] @ wi[e].T, 0.0)
            out[sp] += w_te[sp, e:e + 1] * (h @ wo[e].T)
    return out



# revision 18
# speedup vs baseline: 1.0508x; 1.0322x over previous
"""MoE expert-parallel kernel for Trainium2 (8 NeuronCores).

Problem: nn_Experts (T=8192 tokens, d_model=1024, d_ff=4096, E=8 experts,
top-k=2).  out[t] = sum_e w[t,e] * (relu(x[t] @ wi[e].T) @ wo[e].T), where
w[t,e] is the combined routing weight (0 for unrouted pairs).

Strategy (expert parallelism, sharding_hint):
  - Host: compute w[t,e], gather the tokens routed to each expert e,
    pad to a common capacity C = min(max per-expert count, 1888), and
    ship expert e's tokens + weights to core e.  All-to-all dispatch is
    done host-side; the device kernel is a dense fused 2-layer MLP per
    core.  Tokens beyond the capacity (MoE capacity factor ~0.98) are
    computed exactly on the host in fp32 (~280 of 15.4k pairs, 1.8%).
    Below C=1888 the measured time stops improving (a per-iteration
    floor absorbs further column reductions), so this is the knee.
  - Device (per core, SPMD): yT = wo @ relu(wi @ xT) in a transposed,
    weight-stationary formulation.  Weights are converted to bf16 and kept
    resident in SBUF (16.8 MB); matmuls accumulate in fp32 PSUM; y is
    written back as bf16 (halves writeback traffic; host accumulates fp32).
  - Host: scatter-add w[t,e] * y back into the full [T, d_model] output.

Measured numerics (vs fp32 reference): max-abs rel err ~4.4e-3.

Perf notes (measured on this container's trn2 via slope timing):
  - The kernel is PE-streaming-bound: ~0.53 ns per moving-operand column
    sustained (an effective ~1.9 GHz, below the 2.4 GHz PLL - sustained
    power throttling).  Time is invariant to matmul instruction count at
    fixed total columns (TC=256 == TC=512), so per-instruction overheads
    (LDWEIGHTS, NX dispatch, semaphores) are fully hidden.
  - fp8 paths are dead ends here: e4m3 DoubleRow nets only ~1.2-1.4x
    per MAC sustained and raw-fp8 numerics (5.2e-2) blow the 2e-2 gate;
    error-compensated variants cost more columns than they save.  An
    e3m4 moving operand for mm2 measured *slower* (+1.8%) with 4x the
    error.  A same-weights probe (no LDW traffic) measured no gain.
"""
import os
import sys
from contextlib import ExitStack

import numpy as np

sys.path.insert(0, "/opt/trn_rl_repo")

import concourse.bass as bass
import concourse.mybir as mybir
from concourse import tile
from concourse import bass2jax
from concourse.bass2jax import _bass_exec_p, install_neuronx_cc_hook

T, D_MODEL, D_FF, N_EXPERTS, TOP_K = 8192, 1024, 4096, 8, 2
N_CORES = 8
P = 128          # partitions
TC = int(os.environ.get("KTC", "512"))  # token chunk (matmul moving free dim)
KD = D_MODEL // P    # 8 contraction steps for mm1 / output tiles for mm2
MF = D_FF // P       # 32 ff tiles
COMPUTE_DT = mybir.dt.bfloat16
MM_COL_SPLIT = 1   # 1 = plain [K=128,M=128] matmuls; 2/4 = col-group split
# y writeback dtype: bf16 halves PSUM->SBUF->DRAM traffic; host accumulates
# in fp32 so the extra rounding is ~1 ulp of bf16 on y only.
Y_DT = (mybir.dt.bfloat16 if os.environ.get("KYDT", "bf16") == "bf16"
        else mybir.dt.float32)
# device token capacity cap (capacity factor ~1.0 = T*TOP_K/N_EXPERTS):
# tokens beyond this per expert are computed on the host in fp32 (exact).
# 0 = no cap.
CAPACITY = int(os.environ.get("KCAP", "1888"))
# timing-only probe: all matmuls reuse one weight tile (results are wrong);
# isolates the cost of LDWEIGHTS traffic.
SAME_W = int(os.environ.get("KSAMEW", "0"))
# h (mm1 output / mm2 moving operand) dtype: bfloat16 or float8e3 (e3m4).
H_DT = getattr(mybir.dt, os.environ.get("KHDT", "bfloat16"))


def split_multi_waits(nc, max_waits=1):
    """This container's walrus codegen rejects instructions carrying more
    than a couple of semaphore waits (e.g. the TileContext tail Drain).
    Move excess waits onto preceding NoOps on the same engine."""
    for f in nc.m.functions:
        for b in f.blocks:
            il = b.instructions
            i = 0
            while i < len(il):
                inst = il[i]
                si = inst.sync_info
                if si is not None and len(si.on_wait) > max_waits:
                    waits = list(si.on_wait)
                    si.on_wait = waits[:max_waits]
                    inst.sync_info = si
                    pre = []
                    rest = waits[max_waits:]
                    for k in range(0, len(rest), max_waits):
                        nop = mybir.InstNoOp(
                            name=f"{inst.name}-ws-{k}", ins=[], outs=[])
                        nop.engine = inst.engine
                        nop.sync_info = mybir.SyncInfo(
                            on_wait=rest[k:k + max_waits], on_update=[])
                        pre.append(nop)
                    for n in reversed(pre):
                        il.insert(i, n)
                    i += len(pre)
                i += 1


class SpmdRunner:
    """Compile a Bass program once; run it SPMD on n_cores via PJRT/axon."""

    def __init__(self, nc, n_cores):
        import jax
        from jax.sharding import Mesh, PartitionSpec
        from jax.experimental.shard_map import shard_map

        install_neuronx_cc_hook()
        self.nc = nc
        self.n_cores = n_cores
        partition_name = (nc.partition_id_tensor.name
                          if nc.partition_id_tensor else None)
        in_names, out_names, out_avals, zero_outs = [], [], [], []
        for alloc in nc.m.functions[0].allocations:
            if not isinstance(alloc, mybir.MemoryLocationSet):
                continue
            name = alloc.memorylocations[0].name
            if alloc.kind == "ExternalInput":
                if name != partition_name:
                    in_names.append(name)
            elif alloc.kind == "ExternalOutput":
                out_names.append(name)
                shape = tuple(alloc.tensor_shape)
                dtype = mybir.dt.np(alloc.dtype)
                out_avals.append(jax.core.ShapedArray(shape, dtype))
                zero_outs.append(np.zeros(shape, dtype))
        self.in_names = in_names
        self.out_names = out_names
        self.out_avals = out_avals
        self.zero_outs = zero_outs
        n_params = len(in_names)
        n_outs = len(out_avals)
        all_in_names = list(in_names) + list(out_names)
        if partition_name is not None:
            all_in_names.append(partition_name)
        donate = tuple(range(n_params, n_params + n_outs))

        def _body(*args):
            operands = list(args)
            if partition_name is not None:
                operands.append(bass2jax.partition_id_tensor())
            outs = _bass_exec_p.bind(
                *operands,
                out_avals=tuple(out_avals),
                in_names=tuple(all_in_names),
                out_names=tuple(out_names),
                lowering_input_output_aliases=(),
                sim_require_finite=True,
                sim_require_nnan=True,
                nc=nc,
            )
            return tuple(outs)

        devices = jax.devices()[:n_cores]
        assert len(devices) == n_cores, (
            f"need {n_cores} neuron cores, found {len(jax.devices())}")
        mesh = Mesh(np.asarray(devices), ("core",))
        self.mesh = mesh
        in_specs = (PartitionSpec("core"),) * (n_params + n_outs)
        out_specs = (PartitionSpec("core"),) * n_outs
        self.sharded = jax.jit(
            shard_map(_body, mesh=mesh, in_specs=in_specs,
                      out_specs=out_specs, check_rep=False),
            donate_argnums=donate, keep_unused=True)

    def prep(self, in_maps):
        n = self.n_cores
        concat_in = [
            np.concatenate([np.asarray(in_maps[c][name]) for c in range(n)],
                           axis=0)
            for name in self.in_names
        ]
        concat_zeros = self.device_zeros()
        return concat_in, concat_zeros

    def device_zeros(self):
        """Donated output buffers, created directly on device (no H2D)."""
        import jax
        import jax.numpy as jnp
        from jax.sharding import NamedSharding, PartitionSpec
        if not hasattr(self, "_zeros_fn"):
            n = self.n_cores
            shapes = [(n * z.shape[0], *z.shape[1:]) for z in self.zero_outs]
            dts = [z.dtype for z in self.zero_outs]
            sh = tuple(NamedSharding(self.mesh, PartitionSpec("core"))
                       for _ in shapes)
            self._zeros_fn = jax.jit(
                lambda: tuple(jnp.zeros(s, d) for s, d in zip(shapes, dts)),
                out_shardings=sh)
        return list(self._zeros_fn())

    def run_prepped(self, concat_in, concat_zeros=None):
        if concat_zeros is None:
            concat_zeros = self.device_zeros()
        return self.sharded(*concat_in, *concat_zeros)

    def __call__(self, in_maps):
        out_arrs = self.run_prepped(*self.prep(in_maps))
        n = self.n_cores
        return [
            {name: np.asarray(out_arrs[i]).reshape(
                n, *self.out_avals[i].shape)[c]
             for i, name in enumerate(self.out_names)}
            for c in range(n)
        ]


def chunk_widths(C):
    """Split C tokens into near-equal chunks of width <= TC (multiple of 8),
    to avoid padding the capacity all the way up to a TC multiple."""
    n = -(-C // TC)
    w = -(-(-(-C // n)) // 8) * 8
    widths = [w] * (n - 1) + [C - w * (n - 1)]
    assert all(0 < x <= TC for x in widths) and sum(widths) == C, (C, widths)
    return widths


def build_nc(C, n_repeat=1):
    """Per-core fused MLP: yT[:, :C] = wo @ relu(wi @ xT[:, :C]).

    Inputs (per core): xT [D_MODEL, C] bf16, wiT [D_MODEL, D_FF] bf16
    (= wi[e].T), woT [D_FF, D_MODEL] bf16 (= wo[e].T).
    Output: yT [D_MODEL, C] fp32.
    n_repeat>1 wraps the token-chunk sweep in a hardware loop (for slope
    timing; the result is identical each iteration)."""
    assert C % 8 == 0
    widths = chunk_widths(C)
    starts = [sum(widths[:i]) for i in range(len(widths))]
    nchunk = len(widths)
    TCW = widths[0]
    nc = bass.Bass()
    # x is packed chunk-major by the host: chunk c occupies rows
    # [c*D_MODEL, (c+1)*D_MODEL), columns [0, widths[c]) — every per-tile
    # DMA is then a contiguous block instead of C-strided lines.
    xT = nc.declare_dram_parameter("xT", [nchunk * D_MODEL, TCW],
                                   COMPUTE_DT, isOutput=False)
    wiT = nc.declare_dram_parameter("wiT", [D_MODEL, D_FF], COMPUTE_DT,
                                    isOutput=False)
    woT = nc.declare_dram_parameter("woT", [D_FF, D_MODEL], COMPUTE_DT,
                                    isOutput=False)
    yT = nc.declare_dram_parameter("yT", [D_MODEL, C], Y_DT, isOutput=True)

    with ExitStack() as ctx:
        tc = ctx.enter_context(tile.TileContext(nc))
        wpool = ctx.enter_context(tc.tile_pool(name="w", bufs=1))
        xpool = ctx.enter_context(tc.tile_pool(name="x", bufs=2))
        hpool = ctx.enter_context(tc.tile_pool(name="h", bufs=1))
        ypool = ctx.enter_context(tc.tile_pool(name="y", bufs=4))
        pspool = ctx.enter_context(
            tc.tile_pool(name="ps", bufs=8, space="PSUM"))

        # x for chunk 0 first, then wi in ff-quarters (mm1's m-groups need
        # quarter m//8 only), then wo — so the first matmuls start after
        # ~3 MB of DMA instead of the full 16.8 MB of weights.
        def x_dma(t, c, k, w):
            r0 = c * D_MODEL + k * P
            nc.sync.dma_start(out=t[:], in_=xT[r0:r0 + P, 0:w])

        x0_t = []
        for k in range(KD):
            t = xpool.tile([P, widths[0]], COMPUTE_DT, tag=f"x{k}")
            x_dma(t, 0, k, widths[0])
            x0_t.append(t)
        # first 128 ff-columns of wi separately (m=0 starts after 1.25 MB of
        # DMA instead of 3 MB), then the quarters
        wi_first = []
        for k in range(KD):
            t = wpool.tile([P, P], COMPUTE_DT, tag=f"wif{k}")
            nc.sync.dma_start(out=t[:], in_=wiT[k * P:(k + 1) * P, 0:P])
            wi_first.append(t)
        NQ = 4
        QF = D_FF // NQ
        wi_t = [[None] * NQ for _ in range(KD)]
        for q in range(NQ):
            for k in range(KD):
                t = wpool.tile([P, QF], COMPUTE_DT, tag=f"wi{k}_{q}")
                nc.sync.dma_start(
                    out=t[:], in_=wiT[k * P:(k + 1) * P,
                                      q * QF:(q + 1) * QF])
                wi_t[k][q] = t
        wo_t = []
        for m in range(MF):
            t = wpool.tile([P, D_MODEL], COMPUTE_DT, tag=f"wo{m}")
            nc.sync.dma_start(out=t[:], in_=woT[m * P:(m + 1) * P, :])
            wo_t.append(t)

        def chunk_sweep(first=False):
            for c, (c0, w) in enumerate(zip(starts, widths)):
                sl = slice(c0, c0 + w)
                if first and c == 0:
                    x_t = x0_t
                else:
                    x_t = []
                    for k in range(KD):
                        t = xpool.tile([P, w], COMPUTE_DT, tag=f"x{k}")
                        x_dma(t, c, k, w)
                        x_t.append(t)
                def mm(ps, lhsT_full, rhs, start, stop):
                    if MM_COL_SPLIT == 1:
                        nc.tensor.matmul(ps[:], lhsT_full, rhs,
                                         start=start, stop=stop)
                    else:
                        MS = P // MM_COL_SPLIT
                        for s in range(MM_COL_SPLIT):
                            nc.tensor.matmul(
                                ps[s * MS:(s + 1) * MS, :],
                                lhsT_full[:, s * MS:(s + 1) * MS], rhs,
                                start=start, stop=stop,
                                tile_position=(0, s * MS))

                h_t = []
                for m in range(MF):
                    q, mq = divmod(m, MF // NQ)
                    ps = pspool.tile([P, w], mybir.dt.float32, tag="ps")
                    for k in range(KD):
                        if SAME_W:
                            lhsT = wi_first[0][:]
                        elif first and c == 0 and m == 0:
                            lhsT = wi_first[k][:]
                        else:
                            lhsT = wi_t[k][q][:, mq * P:(mq + 1) * P]
                        mm(ps, lhsT, x_t[k][:], k == 0, k == KD - 1)
                    h = hpool.tile([P, w], H_DT, tag=f"h{m}")
                    nc.scalar.activation(h[:], ps[:],
                                         mybir.ActivationFunctionType.Relu)
                    h_t.append(h)
                for n in range(KD):
                    ps = pspool.tile([P, w], mybir.dt.float32, tag="ps")
                    for m in range(MF):
                        mm(ps,
                           wo_t[0][:, 0:P] if SAME_W else
                           wo_t[m][:, n * P:(n + 1) * P], h_t[m][:],
                           m == 0, m == MF - 1)
                    y = ypool.tile([P, w], Y_DT, tag="y")
                    nc.vector.tensor_copy(y[:], ps[:])
                    nc.sync.dma_start(out=yT[n * P:(n + 1) * P, sl],
                                      in_=y[:])

        if n_repeat == 1:
            chunk_sweep(first=True)
        else:
            with tc.For_i(0, n_repeat, 1,
                          hint_engines=(mybir.EngineType.PE,)):
                chunk_sweep()

    split_multi_waits(nc)
    return nc


_RUNNERS = {}


def _get_runner(C, n_repeat=1):
    key = (C, n_repeat)
    if key not in _RUNNERS:
        _RUNNERS[key] = SpmdRunner(build_nc(C, n_repeat), N_CORES)
    return _RUNNERS[key]


def _route(hidden_states, selected_experts, routing_weights):
    """Combined per-token weight for each expert and per-expert token lists."""
    mask = selected_experts.astype(np.float32)          # [T, K, E]
    w_te = np.einsum('tke,tk->te', mask, routing_weights.astype(np.float32))
    idx = [np.nonzero(w_te[:, e] > 0)[0] for e in range(N_EXPERTS)]
    return w_te, idx


def to_bf16(a):
    """Vectorized fp32 -> bf16 cast (round-to-nearest-even), ~3x faster
    than ml_dtypes astype.  Matches ml_dtypes/hardware rounding for finite
    values (inputs here are well-scaled gaussians)."""
    import ml_dtypes
    a = np.ascontiguousarray(a, dtype=np.float32)
    u = a.view(np.uint32)
    r = ((u + 0x7FFF + ((u >> 16) & 1)) >> 16).astype(np.uint16)
    return r.view(ml_dtypes.bfloat16).reshape(a.shape)


def pack_x(hidden_states, ie, C):
    """Per-core x, chunk-major: [nchunk*D_MODEL, widths[0]] bf16 so every
    in-kernel x DMA is a contiguous block."""
    import ml_dtypes
    widths = chunk_widths(C)
    TCW = widths[0]
    xg = to_bf16(hidden_states[ie].transpose(1, 0))      # [D_MODEL, n]
    out = np.zeros((len(widths) * D_MODEL, TCW), dtype=ml_dtypes.bfloat16)
    c0 = 0
    for c, w in enumerate(widths):
        seg = xg[:, c0:min(c0 + w, xg.shape[1])]
        out[c * D_MODEL:(c + 1) * D_MODEL, :seg.shape[1]] = seg
        c0 += w
    return out


# Cached device-resident weight uploads: [(wi_copy, wo_copy, C, dev_arrays)]
_WEIGHT_CACHE = []


def _pack_weights(wi, wo, runner):
    """bf16-pack and upload the per-expert transposed weights once; reuse the
    device arrays on later calls with identical weights."""
    import jax
    for cwi, cwo, cC, dev in _WEIGHT_CACHE:
        if cC == runner.key_C and np.array_equal(cwi, wi) and \
                np.array_equal(cwo, wo):
            return dev
    wiT = np.concatenate(
        [to_bf16(np.ascontiguousarray(wi[e].transpose(1, 0)))
         for e in range(N_EXPERTS)], axis=0)      # [8*1024, 4096]
    woT = np.concatenate(
        [to_bf16(np.ascontiguousarray(wo[e].transpose(1, 0)))
         for e in range(N_EXPERTS)], axis=0)      # [8*4096, 1024]
    dev = {"wiT": jax.device_put(wiT), "woT": jax.device_put(woT)}
    jax.block_until_ready(list(dev.values()))
    _WEIGHT_CACHE.append((wi.copy(), wo.copy(), runner.key_C, dev))
    del _WEIGHT_CACHE[:-2]
    return dev


def _capacity(idx):
    """Device token capacity: the max per-expert count, rounded up to 8,
    optionally clipped to CAPACITY (overflow tokens go to the host)."""
    max_count = max(len(i) for i in idx)
    C = max(8, ((max_count + 7) // 8) * 8)
    if CAPACITY:
        C = min(C, max(8, ((CAPACITY + 7) // 8) * 8))
    return C


def kernel(hidden_states, selected_experts, routing_weights, wi, wo):
    hidden_states = np.asarray(hidden_states)
    selected_experts = np.asarray(selected_experts)
    routing_weights = np.asarray(routing_weights)
    wi = np.asarray(wi)
    wo = np.asarray(wo)

    w_te, idx = _route(hidden_states, selected_experts, routing_weights)
    C = _capacity(idx)
    idx_dev = [ie[:C] for ie in idx]
    idx_spill = [ie[C:] for ie in idx]
    runner = _get_runner(C)
    runner.key_C = C
    wdev = _pack_weights(wi, wo, runner)

    xT = np.concatenate(
        [pack_x(hidden_states, idx_dev[e], C) for e in range(N_EXPERTS)],
        axis=0)
    concat_in = [{"xT": xT, "wiT": wdev["wiT"], "woT": wdev["woT"]}[name]
                 for name in runner.in_names]

    out_arrs = runner.run_prepped(concat_in)
    yT_all = np.asarray(out_arrs[0]).astype(np.float32).reshape(
        N_EXPERTS, D_MODEL, C)

    out = np.zeros((T, D_MODEL), dtype=np.float32)
    for e in range(N_EXPERTS):
        ie = idx_dev[e]
        out[ie] += w_te[ie, e:e + 1] * yT_all[e, :, :len(ie)].T
        sp = idx_spill[e]
        if len(sp):
            h = np.maximum(hidden_states[sp] @ wi[e].T, 0.0)
            out[sp] += w_te[sp, e:e + 1] * (h @ wo[e].T)
    return out

